# revision 1
# baseline (speedup 1.0000x reference)
"""GraphSAGE link predictor on 8 Trainium2 NeuronCores (Bass/Tile).

Strategy (graph/data parallel, hardcoded from the sharding hint):
- Nodes are sharded contiguously across 8 cores (12500 real -> 12544 padded
  per core, 98 tiles of 128). Edges are sharded by dst node and sorted by dst,
  padded to B 128-edge blocks per dst tile (uniform across cores for SPMD).
- Per layer, per core: h lives transposed in SBUF as hT [128 feat, 12544 node].
  The full row-major h replica lives in DRAM (layer 0: host-supplied x; later
  layers: AllGather of per-core slabs). Per 128-edge block: indirect-DMA gather
  of the 128 source rows ([128,1] offsets -> [128 edge, 128 feat] SBUF tile),
  one-hot matmul scatter (lhsT=onehot, rhs=gathered rows) accumulating
  agg[dst,feat] in PSUM; degree-normalize via per-partition activation scale;
  PE-transpose to aggT[feat,dst]; combine matmuls
  (wn^T @ aggT + ws^T @ hT [+ I @ hT residual]) + bias/relu; transpose back to
  rows, DMA to the slab, AllGather -> replica for the next layer's gathers.
- Decoder: label edges sharded 25000/core; gather h3[src], h3[dst] rows from
  the h3 replica, multiply + reduce -> logits.
All weights ([128,128]) are replicated to every core.
"""

import numpy as np

import concourse.bass as bass
import concourse.bacc as bacc
import concourse.mybir as mybir
import concourse.tile as tile
from concourse.masks import make_identity

P = 128
D = 128
F32 = mybir.dt.float32
I32 = mybir.dt.int32


class Cfg:
    def __init__(self, C, NPCR, B, ELC):
        self.C = C                      # cores
        self.NPCR = NPCR                # real nodes per core
        self.TPC = (NPCR + P - 1) // P  # dst tiles per core
        self.NPC = self.TPC * P         # padded nodes per core
        self.NPAD = C * self.NPC        # padded total nodes
        self.B = B                      # edge blocks per dst tile (uniform)
        self.ELC = ELC                  # label edges per core (real)
        self.LBLK = (ELC + P - 1) // P  # label blocks per core


def build_nc(cfg, n_layers=3, decoder=True, scratch=16384):
    C, TPC, NPC, NPAD, B, LBLK = (
        cfg.C, cfg.TPC, cfg.NPC, cfg.NPAD, cfg.B, cfg.LBLK)
    NBLK = TPC * B

    nc = bacc.Bacc("TRN2", target_bir_lowering=False, debug=False, num_devices=C,
                   dynamic_dma_scratch_size=scratch)

    # ---- I/O ----
    xfull = nc.dram_tensor("xfull", [NPAD, D], F32, kind="ExternalInput")
    xt = nc.dram_tensor("xt", [P, NPC], F32, kind="ExternalInput")
    goff = nc.dram_tensor("goff", [P, NBLK], I32, kind="ExternalInput")
    dloc = nc.dram_tensor("dloc", [P, NBLK], F32, kind="ExternalInput")
    invd = nc.dram_tensor("invd", [P, TPC], F32, kind="ExternalInput")
    wn_d = nc.dram_tensor("wn", [3, D, D], F32, kind="ExternalInput")
    ws_d = nc.dram_tensor("ws", [3, D, D], F32, kind="ExternalInput")
    bias_d = nc.dram_tensor("bias", [3, D], F32, kind="ExternalInput")
    lso = nc.dram_tensor("lso", [P, LBLK], I32, kind="ExternalInput")
    ldo = nc.dram_tensor("ldo", [P, LBLK], I32, kind="ExternalInput")
    logits = nc.dram_tensor("logits", [LBLK, P], F32, kind="ExternalOutput")

    # internal DRAM: per-layer slabs + allgather replicas
    slabs = [nc.dram_tensor(f"slab{l}", [NPC, D], F32, kind="Internal")
             for l in range(3)]
    reps = [nc.dram_tensor(f"rep{l}", [NPAD, D], F32, kind="Internal",
                           addr_space="Shared") for l in range(3)]

    with tile.TileContext(nc) as tc:
        with (
            tc.tile_pool(name="big", bufs=1) as bigp,
            tc.tile_pool(name="const", bufs=1) as cstp,
            tc.tile_pool(name="oh", bufs=1) as ohp,
            tc.tile_pool(name="xe", bufs=1) as xep,
            tc.tile_pool(name="aggsb", bufs=1) as asbp,
            tc.tile_pool(name="rows", bufs=1) as rowp,
            tc.tile_pool(name="psA", bufs=4, space="PSUM") as psA,
            tc.tile_pool(name="psT", bufs=2, space="PSUM") as psT,
            tc.tile_pool(name="psC", bufs=2, space="PSUM") as psC,
        ):
            # ---- constants / static data in SBUF ----
            hA = bigp.tile([P, NPC], F32, name="hA")
            hB = bigp.tile([P, NPC], F32, name="hB")
            goff_sb = cstp.tile([P, NBLK], I32, name="goff_sb")
            dloc_sb = cstp.tile([P, NBLK], F32, name="dloc_sb")
            invd_sb = cstp.tile([P, TPC], F32, name="invd_sb")
            lso_sb = cstp.tile([P, LBLK], I32, name="lso_sb")
            ldo_sb = cstp.tile([P, LBLK], I32, name="ldo_sb")
            cst = cstp.tile([P, 8 * D], F32, name="cst")
            bias_sb = cstp.tile([P, 3], F32, name="bias_sb")

            nc.sync.dma_start(out=hA[:], in_=xt.ap())
            nc.sync.dma_start(out=goff_sb[:], in_=goff.ap())
            nc.sync.dma_start(out=dloc_sb[:], in_=dloc.ap())
            nc.sync.dma_start(out=invd_sb[:], in_=invd.ap())
            nc.sync.dma_start(out=lso_sb[:], in_=lso.ap())
            nc.sync.dma_start(out=ldo_sb[:], in_=ldo.ap())

            ident = cst[:, 0:D]
            iota_f = cst[:, D:2 * D]
            wn_t = [cst[:, (2 + l) * D:(3 + l) * D] for l in range(3)]
            ws_t = [cst[:, (5 + l) * D:(6 + l) * D] for l in range(3)]
            make_identity(nc, ident)
            iota_i = cstp.tile([P, D], I32, name="iota_i")
            nc.gpsimd.iota(iota_i[:], pattern=[[1, D]], base=0,
                           channel_multiplier=0)
            nc.vector.tensor_copy(iota_f, iota_i[:])
            for l in range(3):
                nc.sync.dma_start(out=wn_t[l], in_=wn_d.ap()[l])
                nc.sync.dma_start(out=ws_t[l], in_=ws_d.ap()[l])
                nc.sync.dma_start(out=bias_sb[:, l:l + 1],
                                  in_=bias_d.ap()[l][:, None])

            oh_big = ohp.tile([P, 2 * B * D], F32, name="oh_big")
            xe_big = xep.tile([P, 32 * D], F32, name="xe_big")
            aggsb = asbp.tile([P, 2 * D], F32, name="aggsb")
            aggT = asbp.tile([P, 2 * 512], F32, name="aggT")
            rows_sb = rowp.tile([P, 2 * D], F32, name="rows_sb")

            # ================= 3 GraphSAGE layers =================
            for l in range(n_layers):
                h_in = hA if l % 2 == 0 else hB
                h_out = hB if l % 2 == 0 else hA
                src_t = xfull if l == 0 else reps[l - 1]
                relu = l < 2
                residual = l > 0

                n_chunk = (TPC + 3) // 4
                for ch in range(n_chunk):
                    t0 = ch * 4
                    tiles = list(range(t0, min(t0 + 4, TPC)))
                    cn = len(tiles) * P
                    aggT_c = aggT[:, (ch % 2) * 512:(ch % 2) * 512 + cn]
                    for k, t in enumerate(tiles):
                        # one-hot for all B blocks of tile t in one DVE op
                        oh_t = oh_big[:, (t % 2) * B * D:((t % 2) + 1) * B * D]
                        nc.vector.tensor_tensor(
                            out=oh_t.rearrange("p (b d) -> p b d", b=B),
                            in0=dloc_sb[:, t * B:(t + 1) * B][:, :, None]
                                .broadcast_to([P, B, D]),
                            in1=iota_f[:, None, :].broadcast_to([P, B, D]),
                            op=mybir.AluOpType.is_equal,
                        )
                        agg_ps = psA.tile([P, D], F32, tag="agg",
                                          name=f"agg_{l}_{t}")
                        for b in range(B):
                            xe = xe_big[:, ((t * B + b) % 32) * D:(((t * B + b) % 32) + 1) * D]
                            nc.gpsimd.indirect_dma_start(
                                out=xe,
                                out_offset=None,
                                in_=src_t.ap(),
                                in_offset=bass.IndirectOffsetOnAxis(
                                    ap=goff_sb[:, t * B + b:t * B + b + 1],
                                    axis=0),
                            )
                            nc.tensor.matmul(
                                out=agg_ps[:],
                                lhsT=oh_t[:, b * D:(b + 1) * D],
                                rhs=xe,
                                start=(b == 0), stop=(b == B - 1),
                            )
                        # mean-normalize (per-partition scale) -> [dst, f]
                        agg_n = aggsb[:, (t % 2) * D:((t % 2) + 1) * D]
                        nc.scalar.activation(
                            out=agg_n, in_=agg_ps[:],
                            func=mybir.ActivationFunctionType.Copy,
                            scale=invd_sb[:, t:t + 1],
                        )
                        # transpose -> aggT [f, dst]
                        tps = psT.tile([P, D], F32, tag="tp", name=f"tp_{l}_{t}")
                        nc.tensor.transpose(out=tps[:], in_=agg_n,
                                            identity=ident)
                        nc.vector.tensor_copy(aggT_c[:, k * P:(k + 1) * P],
                                              tps[:])
                    # ---- combine: hnextT = wn^T aggT + ws^T hT (+ hT) ----
                    cs = t0 * P
                    cps = psC.tile([P, 512], F32, tag="comb", name=f"cb_{l}_{ch}")
                    nc.tensor.matmul(out=cps[:, :cn], lhsT=wn_t[l],
                                     rhs=aggT_c, start=True, stop=False)
                    nc.tensor.matmul(out=cps[:, :cn], lhsT=ws_t[l],
                                     rhs=h_in[:, cs:cs + cn],
                                     start=False, stop=not residual)
                    if residual:
                        nc.tensor.matmul(out=cps[:, :cn], lhsT=ident,
                                         rhs=h_in[:, cs:cs + cn],
                                         start=False, stop=True)
                    if relu:
                        nc.scalar.activation(
                            out=h_out[:, cs:cs + cn], in_=cps[:, :cn],
                            func=mybir.ActivationFunctionType.Relu,
                            bias=bias_sb[:, l:l + 1],
                        )
                    else:
                        nc.vector.tensor_scalar_add(
                            out=h_out[:, cs:cs + cn], in0=cps[:, :cn],
                            scalar1=bias_sb[:, l:l + 1],
                        )
                    # ---- rows out to slab for allgather ----
                    for k, t in enumerate(tiles):
                        rps = psT.tile([P, D], F32, tag="tp", name=f"rw_{l}_{t}")
                        nc.tensor.transpose(
                            out=rps[:], in_=h_out[:, t * P:(t + 1) * P],
                            identity=ident)
                        rsb = rows_sb[:, (t % 2) * D:((t % 2) + 1) * D]
                        nc.scalar.activation(
                            out=rsb, in_=rps[:],
                            func=mybir.ActivationFunctionType.Copy)
                        nc.sync.dma_start(
                            out=slabs[l].ap()[t * P:(t + 1) * P, :], in_=rsb)
                nc.gpsimd.collective_compute(
                    "AllGather", mybir.AluOpType.bypass,
                    replica_groups=[list(range(C))],
                    ins=[slabs[l].ap()], outs=[reps[l].ap()],
                )

            # ================= link decoder =================
            DEC = decoder
            lbuf = rowp.tile([P, 24 * D], F32, name="lbuf")
            resL = rowp.tile([P, LBLK], F32, name="resL")
            if not DEC:
                nc.gpsimd.memset(resL[:], 0.0)
            for b in range(LBLK if DEC else 0):
                g0 = (b % 8) * 3 * D
                se = lbuf[:, g0:g0 + D]
                de = lbuf[:, g0 + D:g0 + 2 * D]
                pr = lbuf[:, g0 + 2 * D:g0 + 3 * D]
                nc.gpsimd.indirect_dma_start(
                    out=se, out_offset=None, in_=reps[n_layers - 1].ap(),
                    in_offset=bass.IndirectOffsetOnAxis(
                        ap=lso_sb[:, b:b + 1], axis=0))
                nc.gpsimd.indirect_dma_start(
                    out=de, out_offset=None, in_=reps[n_layers - 1].ap(),
                    in_offset=bass.IndirectOffsetOnAxis(
                        ap=ldo_sb[:, b:b + 1], axis=0))
                nc.vector.tensor_mul(out=pr, in0=se, in1=de)
                nc.vector.tensor_reduce(
                    out=resL[:, b:b + 1], in_=pr,
                    axis=mybir.AxisListType.X, op=mybir.AluOpType.add)
            # transpose resL -> logits rows (block-major)
            nb_full = LBLK // P  # full 128-col transpose blocks
            rem = LBLK - nb_full * P
            for j in range(nb_full + (1 if rem else 0)):
                w = P if j < nb_full else rem
                tps = psT.tile([P, D], F32, tag="tp", name=f"lt_{j}")
                nc.tensor.transpose(out=tps[:w, :],
                                    in_=resL[:, j * P:j * P + w],
                                    identity=ident)
                lsb = rowp.tile([P, D], F32, name=f"lsb_{j}")
                nc.scalar.activation(out=lsb[:w, :], in_=tps[:w, :],
                                     func=mybir.ActivationFunctionType.Copy)
                nc.sync.dma_start(out=logits.ap()[j * P:j * P + w, :],
                                  in_=lsb[:w, :])
    nc.compile()
    return nc


# ----------------------------------------------------------------------------
# host-side preprocessing
# ----------------------------------------------------------------------------

def prep_inputs(cfg, node_features, edge_index, edge_label_index,
                w_neigh, w_self, bias):
    C, NPCR, TPC, NPC, NPAD, B, ELC, LBLK = (
        cfg.C, cfg.NPCR, cfg.TPC, cfg.NPC, cfg.NPAD, cfg.B, cfg.ELC, cfg.LBLK)
    N = node_features.shape[0]
    E = edge_index.shape[1]

    src = np.asarray(edge_index[0], dtype=np.int64)
    dst = np.asarray(edge_index[1], dtype=np.int64)
    deg = np.bincount(dst, minlength=N).astype(np.float32)
    invdeg = 1.0 / np.maximum(deg, 1.0)

    order = np.argsort(dst, kind="stable")
    sdst = dst[order]
    ssrc = src[order]
    c_of = sdst // NPCR
    loc = sdst - c_of * NPCR
    tile_g = c_of * TPC + loc // P
    counts = np.bincount(tile_g, minlength=C * TPC)
    cap = B * P
    assert counts.max() <= cap, (counts.max(), cap)
    starts = np.zeros(C * TPC, np.int64)
    starts[1:] = np.cumsum(counts)[:-1]
    pos = np.arange(E) - starts[tile_g]
    slot = tile_g * cap + pos

    src_pad = np.zeros(C * TPC * cap, np.int32)
    dloc_pad = np.full(C * TPC * cap, -1.0, np.float32)
    src_pidx = (ssrc // NPCR) * NPC + (ssrc % NPCR)
    src_pad[slot] = src_pidx.astype(np.int32)
    dloc_pad[slot] = (loc % P).astype(np.float32)

    # [C, TPC, B, P] -> per core [P, TPC*B] with [p, t*B+b] = arr[t, b, p]
    src_pc = src_pad.reshape(C, TPC, B, P).transpose(0, 3, 1, 2).reshape(
        C, P, TPC * B)
    dloc_pc = dloc_pad.reshape(C, TPC, B, P).transpose(0, 3, 1, 2).reshape(
        C, P, TPC * B)

    # padded x replica
    x = np.asarray(node_features, dtype=np.float32)
    xpad = np.zeros((NPAD, D), np.float32)
    for c in range(C):
        xpad[c * NPC:c * NPC + NPCR] = x[c * NPCR:(c + 1) * NPCR]

    # invd per core: [P, TPC] with [p, t] = invdeg[core, t*128+p]
    invd_pc = np.ones((C, P, TPC), np.float32)
    for c in range(C):
        v = np.ones(NPC, np.float32)
        v[:NPCR] = invdeg[c * NPCR:(c + 1) * NPCR]
        invd_pc[c] = v.reshape(TPC, P).T

    # label edges
    lsrc = np.asarray(edge_label_index[0], dtype=np.int64)
    ldst = np.asarray(edge_label_index[1], dtype=np.int64)
    lsrc_p = ((lsrc // NPCR) * NPC + (lsrc % NPCR)).astype(np.int32)
    ldst_p = ((ldst // NPCR) * NPC + (ldst % NPCR)).astype(np.int32)
    lso_pc = np.zeros((C, P, LBLK), np.int32)
    ldo_pc = np.zeros((C, P, LBLK), np.int32)
    for c in range(C):
        a = np.zeros(LBLK * P, np.int32)
        bb = np.zeros(LBLK * P, np.int32)
        a[:ELC] = lsrc_p[c * ELC:(c + 1) * ELC]
        bb[:ELC] = ldst_p[c * ELC:(c + 1) * ELC]
        lso_pc[c] = a.reshape(LBLK, P).T
        ldo_pc[c] = bb.reshape(LBLK, P).T

    wn = np.ascontiguousarray(np.asarray(w_neigh, dtype=np.float32))
    ws = np.ascontiguousarray(np.asarray(w_self, dtype=np.float32))
    bs = np.ascontiguousarray(np.asarray(bias, dtype=np.float32))

    in_maps = []
    for c in range(C):
        xtc = np.zeros((P, NPC), np.float32)
        xtc[:, :NPCR] = x[c * NPCR:(c + 1) * NPCR].T
        in_maps.append({
            "xfull": xpad,
            "xt": xtc,
            "goff": np.ascontiguousarray(src_pc[c]),
            "dloc": np.ascontiguousarray(dloc_pc[c]),
            "invd": np.ascontiguousarray(invd_pc[c]),
            "wn": wn, "ws": ws, "bias": bs,
            "lso": np.ascontiguousarray(lso_pc[c]),
            "ldo": np.ascontiguousarray(ldo_pc[c]),
        })
    return in_maps


def compute_B(cfg, edge_index):
    dst = np.asarray(edge_index[1], dtype=np.int64)
    c_of = dst // cfg.NPCR
    loc = dst - c_of * cfg.NPCR
    tile_g = c_of * cfg.TPC + loc // P
    counts = np.bincount(tile_g, minlength=cfg.C * cfg.TPC)
    return int((counts.max() + P - 1) // P)


# ----------------------------------------------------------------------------
# PJRT runner (inlined; kernel.py must be self-contained)
# ----------------------------------------------------------------------------

class _Runner:
    def __init__(self, nc, n_cores):
        import jax
        from jax.sharding import Mesh, PartitionSpec
        from jax.experimental.shard_map import shard_map
        from concourse import bass2jax
        from concourse.bass2jax import _bass_exec_p, install_neuronx_cc_hook

        install_neuronx_cc_hook()
        self.jax = jax
        self.n_cores = n_cores
        partition_name = (
            nc.partition_id_tensor.name if nc.partition_id_tensor else None)
        in_names, out_names, out_avals, zero_outs = [], [], [], []
        for alloc in nc.m.functions[0].allocations:
            if not isinstance(alloc, mybir.MemoryLocationSet):
                continue
            name = alloc.memorylocations[0].name
            if alloc.kind == "ExternalInput":
                if name != partition_name:
                    in_names.append(name)
            elif alloc.kind == "ExternalOutput":
                shape = tuple(alloc.tensor_shape)
                dtype = mybir.dt.np(alloc.dtype)
                out_names.append(name)
                out_avals.append(jax.core.ShapedArray(shape, dtype))
                zero_outs.append(np.zeros(shape, dtype))
        self.in_names, self.out_names = in_names, out_names
        self.out_avals, self.zero_outs = out_avals, zero_outs
        all_in = list(in_names) + list(out_names)
        if partition_name is not None:
            all_in.append(partition_name)

        def _body(*args):
            operands = list(args)
            if partition_name is not None:
                operands.append(bass2jax.partition_id_tensor())
            return tuple(_bass_exec_p.bind(
                *operands,
                out_avals=tuple(out_avals),
                in_names=tuple(all_in),
                out_names=tuple(out_names),
                lowering_input_output_aliases=(),
                sim_require_finite=True,
                sim_require_nnan=True,
                nc=nc,
            ))

        devices = jax.devices()[:n_cores]
        self.mesh = Mesh(np.asarray(devices), ("core",))
        n_outs = len(out_names)
        self.fn = jax.jit(
            shard_map(_body, mesh=self.mesh,
                      in_specs=(PartitionSpec("core"),) * (len(in_names) + n_outs),
                      out_specs=(PartitionSpec("core"),) * n_outs,
                      check_rep=False),
            keep_unused=True,
        )

    def stage(self, in_maps):
        from jax.sharding import NamedSharding, PartitionSpec
        concat = [np.concatenate([np.asarray(m[n]) for m in in_maps], axis=0)
                  for n in self.in_names]
        concat += [np.zeros((self.n_cores * z.shape[0], *z.shape[1:]), z.dtype)
                   for z in self.zero_outs]
        sh = NamedSharding(self.mesh, PartitionSpec("core"))
        staged = [self.jax.device_put(a, sh) for a in concat]
        self.jax.block_until_ready(staged)
        return staged

    def run_staged(self, staged):
        outs = self.fn(*staged)
        self.jax.block_until_ready(outs)
        return outs

    def split(self, outs):
        return [
            {n: np.asarray(outs[i]).reshape(self.n_cores,
                                            *self.out_avals[i].shape)[c]
             for i, n in enumerate(self.out_names)}
            for c in range(self.n_cores)
        ]


_CACHE = {}


def _get_runner(cfg_key, cfg):
    if cfg_key not in _CACHE:
        nc = build_nc(cfg)
        _CACHE[cfg_key] = _Runner(nc, cfg.C)
    return _CACHE[cfg_key]


def kernel(node_features, edge_index, edge_label_index, w_neigh, w_self,
           bias):
    node_features = np.asarray(node_features)
    edge_index = np.asarray(edge_index)
    edge_label_index = np.asarray(edge_label_index)
    N = node_features.shape[0]
    C = 8
    NPCR = N // C
    ELC = edge_label_index.shape[1] // C
    cfg = Cfg(C, NPCR, 18, ELC)
    B = compute_B(cfg, edge_index)
    if B > cfg.B:
        cfg = Cfg(C, NPCR, B, ELC)
    runner = _get_runner((C, NPCR, cfg.B, ELC), cfg)
    in_maps = prep_inputs(cfg, node_features, edge_index, edge_label_index,
                          w_neigh, w_self, bias)
    outs = runner.split(runner.run_staged(runner.stage(in_maps)))
    logits = np.concatenate(
        [outs[c]["logits"].reshape(-1)[:ELC] for c in range(C)])
    return logits.astype(np.float32)



# revision 9
# speedup vs baseline: 1.4180x; 1.4180x over previous
"""GraphSAGE link predictor on 8 Trainium2 NeuronCores (Bass/Tile).

Strategy (graph/data parallel, from the sharding hint):
- Nodes are sharded contiguously across 8 cores (12500 real -> 12544 padded
  per core, 98 dst tiles of 128). Edges are sharded by dst node. Features
  travel in bf16 (fp32 accumulation in PSUM); ~3e-3 max rel err vs fp32.
- The full row-major bf16 h replica lives in DRAM (layer 0: host-supplied x;
  later: AllGather of per-core slabs). Edge-source rows are gathered with the
  gpsimd dma_gather ucode (thousands of rows per Pool dispatch vs 128 for
  indirect_dma_start). Its int16 indices cap the table at 32K rows, so the
  replica is split into 4 windows of NPAD/4 rows; per (4-dst-tile chunk,
  window) one gather call fetches every needed source row. Slots are laid out
  [window][tile] with 16-aligned per-(tile,window) capacities (max over cores
  for SPMD uniformity), so a 128-slot block may straddle two adjacent tiles:
  those boundary blocks get a second one-hot (dlocB) and a second matmul.
- Per 128-edge block: one-hot matmul (lhsT=onehot[edge,dst], rhs=rows) accum
  agg[dst,feat] in PSUM; degree-normalize via per-partition ACT scale (bf16);
  PE-transpose to aggT[feat,dst]; combine matmuls (wn^T aggT + ws^T hT
  [+ I hT]) + bias/relu; transpose back to rows, chunked DMA to the slab,
  AllGather -> replica for the next layer.
- Decoder: label-edge pairs grouped by (src window, dst window) into 16
  groups; per group two dma_gathers (h3[src], h3[dst]) + DVE mul/reduce ->
  logits; the host inverts the slot permutation.
All weights ([128,128]) are replicated to every core.
"""

import numpy as np

import concourse.bacc as bacc
import concourse.mybir as mybir
import concourse.tile as tile
from concourse.masks import make_identity

P = 128
D = 128
NW = 4                    # index windows over the padded node table
MAXI = 1024               # max idxs per dma_gather call (desc-ring capacity)
F32 = mybir.dt.float32
BF16 = mybir.dt.bfloat16
I16 = mybir.dt.int16


def _bf(a):
    import ml_dtypes
    return np.asarray(a).astype(ml_dtypes.bfloat16)


def _r16(x):
    return (x + 15) // 16 * 16


class Cfg:
    def __init__(self, C, NPCR, ELC):
        self.C = C
        self.NPCR = NPCR
        self.TPC = (NPCR + P - 1) // P
        self.NPC = self.TPC * P
        self.NPAD = C * self.NPC
        self.ELC = ELC
        assert self.NPAD % NW == 0
        self.WROWS = self.NPAD // NW
        assert self.WROWS <= 32767


class Layout:
    """Core-uniform slot layout (compile-time structure)."""

    def __init__(self, cfg, edge_index, edge_label_index):
        C, TPC, NPCR, NPC, WROWS = (cfg.C, cfg.TPC, cfg.NPCR, cfg.NPC,
                                    cfg.WROWS)
        src = np.asarray(edge_index[0], dtype=np.int64)
        dst = np.asarray(edge_index[1], dtype=np.int64)
        src_pidx = (src // NPCR) * NPC + (src % NPCR)
        win = src_pidx // WROWS
        c_of = dst // NPCR
        loc = dst - c_of * NPCR
        t_of = loc // P
        # counts per (core, tile, window) -> capacities (max over cores, r16)
        cnt = np.zeros((C * TPC, NW), np.int64)
        np.add.at(cnt, (c_of * TPC + t_of, win), 1)
        cnt = cnt.reshape(C, TPC, NW)
        caps = _r16(cnt.max(axis=0))
        caps = np.where((caps > 0) & (caps < P), P, caps)
        dead = caps.sum(axis=1) == 0
        caps[dead, 0] = P            # tiles with no edges anywhere: 1 pad blk
        self.caps = caps

        n_chunk = (TPC + 3) // 4
        self.n_chunk = n_chunk
        self.S = np.zeros((n_chunk, NW), np.int64)      # idxs per (chunk,win)
        self.wb0 = np.zeros((n_chunk, NW), np.int64)    # block base in chunk
        self.BCH = np.zeros(n_chunk, np.int64)          # blocks per chunk
        self.qbase = np.zeros((TPC, NW), np.int64)      # q offset of (t,k)
        for ch in range(n_chunk):
            t0, t1 = ch * 4, min(ch * 4 + 4, TPC)
            nb = 0
            for k in range(NW):
                q = 0
                for t in range(t0, t1):
                    self.qbase[t, k] = q
                    q += caps[t, k]
                self.S[ch, k] = q
                self.wb0[ch, k] = nb
                nb += (q + P - 1) // P
            self.BCH[ch] = nb
        self.BMAX = int(self.BCH.max())
        self.O = np.zeros(n_chunk + 1, np.int64)        # global block col base
        np.cumsum(self.BCH, out=self.O[1:])
        self.TOTB = int(self.O[-1])
        # idx col offsets per (chunk, win)
        self.oc = np.zeros((n_chunk, NW), np.int64)
        o = 0
        for ch in range(n_chunk):
            for k in range(NW):
                self.oc[ch, k] = o
                o += self.S[ch, k] // 16
        self.TOTC = int(o)
        self.ICMAX = int(max(self.S[ch].sum() // 16 for ch in range(n_chunk)))

        # primary/secondary tile per global block col + boundary packing
        self.primary = np.zeros(self.TOTB, np.int64)
        self.bnd_col = np.full(self.TOTB, -1, np.int64)  # packed dlocB col
        sched = []                                      # [ch][t-local] ops
        nbb = 0
        self.bnd_of_chunk = []
        for ch in range(n_chunk):
            t0, t1 = ch * 4, min(ch * 4 + 4, TPC)
            ops = [[] for _ in range(t1 - t0)]
            ch_bnd = 0
            for k in range(NW):
                # tile of flat q (within chunk,win)
                edges = []
                for t in range(t0, t1):
                    edges += [t] * caps[t, k]
                edges = np.asarray(edges, np.int64)
                for b in range((self.S[ch, k] + P - 1) // P):
                    j = self.O[ch] + self.wb0[ch, k] + b
                    q0, q1 = b * P, min(b * P + P, self.S[ch, k])
                    tset = edges[q0:q1]
                    tp = int(tset[0])
                    self.primary[j] = tp
                    ops[tp - t0].append(("A", int(j)))
                    if int(tset[-1]) != tp:
                        ts = int(tset[-1])
                        assert ts == tp + 1, (tp, ts)
                        self.bnd_col[j] = nbb + ch_bnd
                        ops[ts - t0].append(("B", int(nbb + ch_bnd)))
                        ch_bnd += 1
            self.bnd_of_chunk.append((nbb, ch_bnd))
            nbb += ch_bnd
            sched.append(ops)
        self.sched = sched
        self.TOTBB = max(int(nbb), 1)
        self.NBMAX = max(max(b for _, b in self.bnd_of_chunk), 1)

        # ---- decoder groups ----
        lsrc = np.asarray(edge_label_index[0], dtype=np.int64)
        ldst = np.asarray(edge_label_index[1], dtype=np.int64)
        ELC = cfg.ELC
        lc = np.arange(lsrc.shape[0]) // ELC
        ls_p = (lsrc // NPCR) * NPC + (lsrc % NPCR)
        ld_p = (ldst // NPCR) * NPC + (ldst % NPCR)
        gg = (ls_p // WROWS) * NW + (ld_p // WROWS)
        gcnt = np.zeros((C, NW * NW), np.int64)
        np.add.at(gcnt, (lc, gg), 1)
        self.dcaps = _r16(gcnt.max(axis=0))
        self.dblocks = (self.dcaps + P - 1) // P
        self.Gbase = np.zeros(NW * NW + 1, np.int64)
        np.cumsum(self.dblocks, out=self.Gbase[1:])
        self.LBLKP = int(self.Gbase[-1])
        self.DGBMAX = int(self.dblocks.max())
        # decoder idx col offsets (se then de per group), after main idx cols
        self.doc = np.zeros((NW * NW, 2), np.int64)
        o = self.TOTC
        for g in range(NW * NW):
            self.doc[g, 0] = o
            o += self.dcaps[g] // 16
            self.doc[g, 1] = o
            o += self.dcaps[g] // 16
        self.TOTALC = int(o)

    def key(self):
        import hashlib
        h = hashlib.sha1()
        h.update(self.caps.tobytes())
        h.update(self.dcaps.tobytes())
        return h.hexdigest()


def build_nc(cfg, lay, n_layers=3, scratch=16384):
    C, TPC, NPC, NPAD, WROWS = (cfg.C, cfg.TPC, cfg.NPC, cfg.NPAD, cfg.WROWS)
    n_chunk, BMAX, NBMAX = lay.n_chunk, lay.BMAX, lay.NBMAX
    LBLKP, DGBMAX = lay.LBLKP, lay.DGBMAX

    nc = bacc.Bacc("TRN2", target_bir_lowering=False, debug=False,
                   num_devices=C, dynamic_dma_scratch_size=scratch)

    # ---- I/O ----
    xfull = nc.dram_tensor("xfull", [NPAD, D], BF16, kind="ExternalInput")
    xt = nc.dram_tensor("xt", [P, NPC], BF16, kind="ExternalInput")
    idxs = nc.dram_tensor("idxs", [P, lay.TOTALC], I16, kind="ExternalInput")
    dlocA = nc.dram_tensor("dlocA", [P, lay.TOTB], BF16, kind="ExternalInput")
    dlocB = nc.dram_tensor("dlocB", [P, lay.TOTBB], BF16,
                           kind="ExternalInput")
    invd = nc.dram_tensor("invd", [P, TPC], F32, kind="ExternalInput")
    wn_d = nc.dram_tensor("wn", [3, D, D], BF16, kind="ExternalInput")
    ws_d = nc.dram_tensor("ws", [3, D, D], BF16, kind="ExternalInput")
    bias_d = nc.dram_tensor("bias", [3, D], F32, kind="ExternalInput")
    logits = nc.dram_tensor("logits", [LBLKP, P], F32, kind="ExternalOutput")

    slabs = [nc.dram_tensor(f"slab{l}", [NPC, D], BF16, kind="Internal")
             for l in range(3)]
    reps = [nc.dram_tensor(f"rep{l}", [NPAD, D], BF16, kind="Internal",
                           addr_space="Shared") for l in range(3)]

    with tile.TileContext(nc) as tc:
        with (
            tc.tile_pool(name="big", bufs=1) as bigp,
            tc.tile_pool(name="const", bufs=1) as cstp,
            tc.tile_pool(name="oh", bufs=1) as ohp,
            tc.tile_pool(name="xe", bufs=1) as xep,
            tc.tile_pool(name="aggsb", bufs=1) as asbp,
            tc.tile_pool(name="rows", bufs=1) as rowp,
            tc.tile_pool(name="dec", bufs=1) as decp,
            tc.tile_pool(name="psA", bufs=4, space="PSUM") as psA,
            tc.tile_pool(name="psT", bufs=2, space="PSUM") as psT,
            tc.tile_pool(name="psC", bufs=2, space="PSUM") as psC,
        ):
            hA = bigp.tile([P, NPC], BF16, name="hA")
            hB = bigp.tile([P, NPC], BF16, name="hB")
            dlocA_sb = cstp.tile([P, lay.TOTB], BF16, name="dlocA_sb")
            dlocB_sb = cstp.tile([P, lay.TOTBB], BF16, name="dlocB_sb")
            invd_sb = cstp.tile([P, TPC], F32, name="invd_sb")
            bias_sb = cstp.tile([P, 3], F32, name="bias_sb")
            ident_b = cstp.tile([P, D], BF16, name="ident_b")
            ident_f = cstp.tile([P, D], F32, name="ident_f")
            iota_b = cstp.tile([P, D], BF16, name="iota_b")
            wcst = cstp.tile([P, 6 * D], BF16, name="wcst")

            nc.sync.dma_start(out=hA[:], in_=xt.ap())
            nc.sync.dma_start(out=dlocA_sb[:], in_=dlocA.ap())
            nc.sync.dma_start(out=dlocB_sb[:], in_=dlocB.ap())
            nc.sync.dma_start(out=invd_sb[:], in_=invd.ap())

            wn_t = [wcst[:, l * D:(l + 1) * D] for l in range(3)]
            ws_t = [wcst[:, (3 + l) * D:(4 + l) * D] for l in range(3)]
            make_identity(nc, ident_b)
            make_identity(nc, ident_f)
            iota_i = cstp.tile([P, D], mybir.dt.int32, name="iota_i")
            nc.gpsimd.iota(iota_i[:], pattern=[[1, D]], base=0,
                           channel_multiplier=0)
            nc.vector.tensor_copy(iota_b, iota_i[:])
            for l in range(3):
                nc.sync.dma_start(out=wn_t[l], in_=wn_d.ap()[l])
                nc.sync.dma_start(out=ws_t[l], in_=ws_d.ap()[l])
                nc.sync.dma_start(out=bias_sb[:, l:l + 1],
                                  in_=bias_d.ap()[l][:, None])

            xe_big = xep.tile([P, 2 * BMAX * D], BF16, name="xe_big")
            ohA_big = ohp.tile([P, 2 * BMAX * D], BF16, name="ohA_big")
            ohB_big = ohp.tile([P, 2 * NBMAX * D], BF16, name="ohB_big")
            idx_sb = cstp.tile([P, 2 * lay.ICMAX], I16, name="idx_sb")
            aggn = asbp.tile([P, 2 * D], BF16, name="aggn")
            aggT = asbp.tile([P, 2 * 512], BF16, name="aggT")
            rows_sb = rowp.tile([P, 2 * 4 * D], BF16, name="rows_sb")
            nc.vector.memset(xe_big[:], 0.0)

            slab_v = [slabs[l].ap().rearrange("(g p) d -> p g d", p=P)
                      for l in range(3)]

            # ================= 3 GraphSAGE layers =================
            for l in range(n_layers):
                h_in = hA if l % 2 == 0 else hB
                h_out = hB if l % 2 == 0 else hA
                src_t = xfull if l == 0 else reps[l - 1]

                for ch in range(n_chunk):
                    t0 = ch * 4
                    tiles = list(range(t0, min(t0 + 4, TPC)))
                    cn = len(tiles) * P
                    sl = ch % 2
                    # stage this chunk's idx cols
                    c0 = int(lay.oc[ch, 0])
                    ccols = int(lay.S[ch].sum() // 16)
                    ix = idx_sb[:, sl * lay.ICMAX:sl * lay.ICMAX + ccols]
                    nc.sync.dma_start(out=ix, in_=idxs.ap()[:, c0:c0 + ccols])
                    # one gather per window (split at MAXI idxs per call)
                    xe_c = xe_big[:, sl * BMAX * D:(sl + 1) * BMAX * D]
                    for k in range(NW):
                        S = int(lay.S[ch, k])
                        if S == 0:
                            continue
                        b0 = int(lay.wb0[ch, k])
                        o0 = int(lay.oc[ch, k]) - c0
                        for s0 in range(0, S, MAXI):
                            ns = min(MAXI, S - s0)
                            nb_k = (ns + P - 1) // P
                            bb = b0 + s0 // P
                            nc.gpsimd.dma_gather(
                                out_ap=xe_c[:, bb * D:(bb + nb_k) * D]
                                    .rearrange("p (s d) -> p s d", d=D),
                                in_ap=src_t.ap()[k * WROWS:(k + 1) * WROWS],
                                idxs_ap=ix[:, o0 + s0 // 16:
                                           o0 + s0 // 16 + ns // 16],
                                num_idxs=ns, num_idxs_reg=ns, elem_size=D)
                    # one-hots
                    BCH = int(lay.BCH[ch])
                    ohA_c = ohA_big[:, sl * BMAX * D:sl * BMAX * D + BCH * D]
                    nc.vector.tensor_tensor(
                        out=ohA_c.rearrange("p (b d) -> p b d", b=BCH),
                        in0=dlocA_sb[:, lay.O[ch]:lay.O[ch] + BCH][:, :, None]
                            .broadcast_to([P, BCH, D]),
                        in1=iota_b[:, None, :].broadcast_to([P, BCH, D]),
                        op=mybir.AluOpType.is_equal,
                    )
                    bb0, nbnd = lay.bnd_of_chunk[ch]
                    if nbnd:
                        ohB_c = ohB_big[:, sl * NBMAX * D:
                                        sl * NBMAX * D + nbnd * D]
                        nc.vector.tensor_tensor(
                            out=ohB_c.rearrange("p (b d) -> p b d", b=nbnd),
                            in0=dlocB_sb[:, bb0:bb0 + nbnd][:, :, None]
                                .broadcast_to([P, nbnd, D]),
                            in1=iota_b[:, None, :].broadcast_to([P, nbnd, D]),
                            op=mybir.AluOpType.is_equal,
                        )
                    aggT_c = aggT[:, sl * 512:sl * 512 + cn]
                    for kk, t in enumerate(tiles):
                        ops = lay.sched[ch][kk]
                        agg_ps = psA.tile([P, D], F32, tag="agg",
                                          name=f"agg_{l}_{t}")
                        for j, (which, col) in enumerate(ops):
                            if which == "A":
                                oh = ohA_c[:, (col - int(lay.O[ch])) * D:
                                           (col - int(lay.O[ch]) + 1) * D]
                                xe = xe_c[:, (col - int(lay.O[ch])) * D:
                                          (col - int(lay.O[ch]) + 1) * D]
                            else:
                                oh = ohB_c[:, (col - bb0) * D:
                                           (col - bb0 + 1) * D]
                                # find the A col of this boundary block
                                gcol = int(np.where(lay.bnd_col == col)[0][0])
                                xe = xe_c[:, (gcol - int(lay.O[ch])) * D:
                                          (gcol - int(lay.O[ch]) + 1) * D]
                            nc.tensor.matmul(
                                out=agg_ps[:], lhsT=oh, rhs=xe,
                                start=(j == 0), stop=(j == len(ops) - 1),
                            )
                        agg_n = aggn[:, (t % 2) * D:((t % 2) + 1) * D]
                        nc.scalar.activation(
                            out=agg_n, in_=agg_ps[:],
                            func=mybir.ActivationFunctionType.Copy,
                            scale=invd_sb[:, t:t + 1],
                        )
                        tps = psT.tile([P, D], BF16, tag="tp",
                                       name=f"tp_{l}_{t}")
                        nc.tensor.transpose(out=tps[:], in_=agg_n,
                                            identity=ident_b)
                        nc.vector.tensor_copy(aggT_c[:, kk * P:(kk + 1) * P],
                                              tps[:])
                    # combine
                    cs = t0 * P
                    cps = psC.tile([P, 512], F32, tag="comb",
                                   name=f"cb_{l}_{ch}")
                    nc.tensor.matmul(out=cps[:, :cn], lhsT=wn_t[l],
                                     rhs=aggT_c, start=True, stop=False)
                    nc.tensor.matmul(out=cps[:, :cn], lhsT=ws_t[l],
                                     rhs=h_in[:, cs:cs + cn],
                                     start=False, stop=l == 0)
                    if l > 0:
                        nc.tensor.matmul(out=cps[:, :cn], lhsT=ident_b,
                                         rhs=h_in[:, cs:cs + cn],
                                         start=False, stop=True)
                    if l < 2:
                        nc.scalar.activation(
                            out=h_out[:, cs:cs + cn], in_=cps[:, :cn],
                            func=mybir.ActivationFunctionType.Relu,
                            bias=bias_sb[:, l:l + 1],
                        )
                    else:
                        nc.vector.tensor_scalar_add(
                            out=h_out[:, cs:cs + cn], in0=cps[:, :cn],
                            scalar1=bias_sb[:, l:l + 1],
                        )
                    rsb = rows_sb[:, sl * 4 * D:sl * 4 * D + len(tiles) * D]
                    for kk, t in enumerate(tiles):
                        rps = psT.tile([P, D], BF16, tag="tp",
                                       name=f"rw_{l}_{t}")
                        nc.tensor.transpose(
                            out=rps[:], in_=h_out[:, t * P:(t + 1) * P],
                            identity=ident_b)
                        nc.scalar.activation(
                            out=rsb[:, kk * D:(kk + 1) * D], in_=rps[:],
                            func=mybir.ActivationFunctionType.Copy)
                    nc.sync.dma_start(
                        out=slab_v[l][:, t0:t0 + len(tiles), :],
                        in_=rsb.rearrange("p (g d) -> p g d", g=len(tiles)))
                nc.gpsimd.collective_compute(
                    "AllGather", mybir.AluOpType.bypass,
                    replica_groups=[list(range(C))],
                    ins=[slabs[l].ap()], outs=[reps[l].ap()],
                )

            # ================= link decoder =================
            se_big = decp.tile([P, 2 * DGBMAX * D], BF16, name="se_big")
            de_big = decp.tile([P, 2 * DGBMAX * D], BF16, name="de_big")
            didx = decp.tile([P, 2 * 2 * (DGBMAX * P // 16)], I16,
                             name="didx")
            pr = decp.tile([P, DGBMAX * D], F32, name="pr")
            resL = rowp.tile([P, LBLKP], F32, name="resL")
            nc.vector.memset(se_big[:], 0.0)
            nc.vector.memset(de_big[:], 0.0)
            rep3 = reps[n_layers - 1]
            for g in range(NW * NW):
                cap = int(lay.dcaps[g])
                if cap == 0:
                    continue
                gb = int(lay.dblocks[g])
                gb0 = int(lay.Gbase[g])
                sl = g % 2
                ks, kd = g // NW, g % NW
                hw = DGBMAX * P // 16
                icols = cap // 16
                ixs = didx[:, sl * 2 * hw:sl * 2 * hw + icols]
                ixd = didx[:, sl * 2 * hw + hw:sl * 2 * hw + hw + icols]
                so = int(lay.doc[g, 0])
                do = int(lay.doc[g, 1])
                nc.sync.dma_start(out=ixs, in_=idxs.ap()[:, so:so + icols])
                nc.sync.dma_start(out=ixd, in_=idxs.ap()[:, do:do + icols])
                se = se_big[:, sl * DGBMAX * D:sl * DGBMAX * D + gb * D]
                de = de_big[:, sl * DGBMAX * D:sl * DGBMAX * D + gb * D]
                for s0 in range(0, cap, MAXI):
                    ns = min(MAXI, cap - s0)
                    nb_k = (ns + P - 1) // P
                    bb = s0 // P
                    nc.gpsimd.dma_gather(
                        out_ap=se[:, bb * D:(bb + nb_k) * D]
                            .rearrange("p (s d) -> p s d", d=D),
                        in_ap=rep3.ap()[ks * WROWS:(ks + 1) * WROWS],
                        idxs_ap=ixs[:, s0 // 16:s0 // 16 + ns // 16],
                        num_idxs=ns, num_idxs_reg=ns, elem_size=D)
                    nc.gpsimd.dma_gather(
                        out_ap=de[:, bb * D:(bb + nb_k) * D]
                            .rearrange("p (s d) -> p s d", d=D),
                        in_ap=rep3.ap()[kd * WROWS:(kd + 1) * WROWS],
                        idxs_ap=ixd[:, s0 // 16:s0 // 16 + ns // 16],
                        num_idxs=ns, num_idxs_reg=ns, elem_size=D)
                nc.vector.tensor_mul(out=pr[:, :gb * D], in0=se, in1=de)
                nc.vector.tensor_reduce(
                    out=resL[:, gb0:gb0 + gb],
                    in_=pr[:, :gb * D].rearrange("p (b d) -> p b d", b=gb),
                    axis=mybir.AxisListType.X, op=mybir.AluOpType.add)
            # transpose resL -> logits rows
            nb_full = LBLKP // P
            rem = LBLKP - nb_full * P
            for j in range(nb_full + (1 if rem else 0)):
                w = P if j < nb_full else rem
                tps = psT.tile([P, D], F32, tag="tp", name=f"lt_{j}")
                nc.tensor.transpose(out=tps[:w, :],
                                    in_=resL[:, j * P:j * P + w],
                                    identity=ident_f)
                lsb = rowp.tile([P, D], F32, name=f"lsb_{j}")
                nc.scalar.activation(out=lsb[:w, :], in_=tps[:w, :],
                                     func=mybir.ActivationFunctionType.Copy)
                nc.sync.dma_start(out=logits.ap()[j * P:j * P + w, :],
                                  in_=lsb[:w, :])
    nc.compile()
    return nc


# ----------------------------------------------------------------------------
# host-side preprocessing
# ----------------------------------------------------------------------------

def prep_inputs(cfg, lay, node_features, edge_index, edge_label_index,
                w_neigh, w_self, bias):
    C, NPCR, TPC, NPC, NPAD, ELC, WROWS = (
        cfg.C, cfg.NPCR, cfg.TPC, cfg.NPC, cfg.NPAD, cfg.ELC, cfg.WROWS)
    N = node_features.shape[0]

    src = np.asarray(edge_index[0], dtype=np.int64)
    dst = np.asarray(edge_index[1], dtype=np.int64)
    deg = np.bincount(dst, minlength=N).astype(np.float32)
    invdeg = 1.0 / np.maximum(deg, 1.0)

    src_pidx = (src // NPCR) * NPC + (src % NPCR)
    win = src_pidx // WROWS
    c_of = dst // NPCR
    loc = dst - c_of * NPCR
    t_of = loc // P
    d_of = loc % P

    # rank of each edge within its (core, tile, window) group
    key = (c_of * TPC + t_of) * NW + win
    order = np.argsort(key, kind="stable")
    skey = key[order]
    gcnt = np.bincount(skey, minlength=C * TPC * NW)
    starts = np.zeros(C * TPC * NW, np.int64)
    starts[1:] = np.cumsum(gcnt)[:-1]
    rank = np.arange(src.shape[0]) - starts[skey]
    # scatter rank back to edge order
    rank_e = np.empty_like(rank)
    rank_e[order] = rank

    ch_of = t_of // 4
    # flat q within (chunk, window)
    q = lay.qbase[t_of, win] + rank_e
    blk_in_ch = lay.wb0[ch_of, win] + q // P       # block col within chunk
    gblk = lay.O[ch_of] + blk_in_ch                # global block col
    part = q % P
    # idx position (within (ch,win) idx cols)
    icol = lay.oc[ch_of, win] + q // 16
    irow = q % 16

    idx_val = (src_pidx - win * WROWS).astype(np.int16)
    prim = lay.primary[gblk]
    is_prim = prim == t_of
    bcol = lay.bnd_col[gblk]

    idxs_pc = np.zeros((C, P, lay.TOTALC), np.int16)
    dlocA_pc = np.full((C, P, lay.TOTB), -1.0, np.float32)
    dlocB_pc = np.full((C, P, lay.TOTBB), -1.0, np.float32)
    for c in range(C):
        m = c_of == c
        for g in range(8):
            idxs_pc[c, 16 * g + irow[m] % 16, icol[m]] = idx_val[m]
        mp = m & is_prim
        dlocA_pc[c, part[mp], gblk[mp]] = d_of[mp]
        ms = m & ~is_prim
        assert (bcol[ms] >= 0).all()
        dlocB_pc[c, part[ms], bcol[ms]] = d_of[ms]

    # padded x replica (bf16)
    x = np.asarray(node_features, dtype=np.float32)
    xpad = np.zeros((NPAD, D), np.float32)
    for c in range(C):
        xpad[c * NPC:c * NPC + NPCR] = x[c * NPCR:(c + 1) * NPCR]
    xpad = _bf(xpad)

    invd_pc = np.ones((C, P, TPC), np.float32)
    for c in range(C):
        v = np.ones(NPC, np.float32)
        v[:NPCR] = invdeg[c * NPCR:(c + 1) * NPCR]
        invd_pc[c] = v.reshape(TPC, P).T

    # ---- decoder ----
    lsrc = np.asarray(edge_label_index[0], dtype=np.int64)
    ldst = np.asarray(edge_label_index[1], dtype=np.int64)
    lc = np.arange(lsrc.shape[0]) // ELC
    ls_p = (lsrc // NPCR) * NPC + (lsrc % NPCR)
    ld_p = (ldst // NPCR) * NPC + (ldst % NPCR)
    ks = ls_p // WROWS
    kd = ld_p // WROWS
    gg = ks * NW + kd
    gkey = lc * (NW * NW) + gg
    gorder = np.argsort(gkey, kind="stable")
    sgkey = gkey[gorder]
    ggcnt = np.bincount(sgkey, minlength=C * NW * NW)
    gstarts = np.zeros(C * NW * NW, np.int64)
    gstarts[1:] = np.cumsum(ggcnt)[:-1]
    grank = np.arange(lsrc.shape[0]) - gstarts[sgkey]
    grank_e = np.empty_like(grank)
    grank_e[gorder] = grank

    # slot -> resL position: col = Gbase[g] + r//128, partition = r%128
    lcol = lay.Gbase[gg] + grank_e // P
    lpart = grank_e % P
    # host-side inverse perm: label i of core c reads logits[lcol, lpart]
    perm = (lcol, lpart)

    se_val = (ls_p - ks * WROWS).astype(np.int16)
    de_val = (ld_p - kd * WROWS).astype(np.int16)
    sicol = lay.doc[gg, 0] + grank_e // 16
    dicol = lay.doc[gg, 1] + grank_e // 16
    for c in range(C):
        m = lc == c
        for g in range(8):
            rr = 16 * g + grank_e[m] % 16
            idxs_pc[c, rr, sicol[m]] = se_val[m]
            idxs_pc[c, rr, dicol[m]] = de_val[m]

    wn = _bf(np.ascontiguousarray(np.asarray(w_neigh, dtype=np.float32)))
    ws = _bf(np.ascontiguousarray(np.asarray(w_self, dtype=np.float32)))
    bs = np.ascontiguousarray(np.asarray(bias, dtype=np.float32))

    in_maps = []
    for c in range(C):
        xtc = np.zeros((P, NPC), np.float32)
        xtc[:, :NPCR] = x[c * NPCR:(c + 1) * NPCR].T
        in_maps.append({
            "xfull": xpad,
            "xt": _bf(xtc),
            "idxs": np.ascontiguousarray(idxs_pc[c]),
            "dlocA": _bf(dlocA_pc[c]),
            "dlocB": _bf(dlocB_pc[c]),
            "invd": np.ascontiguousarray(invd_pc[c]),
            "wn": wn, "ws": ws, "bias": bs,
        })
    return in_maps, perm


# ----------------------------------------------------------------------------
# PJRT runner (inlined; kernel.py must be self-contained)
# ----------------------------------------------------------------------------

class _Runner:
    def __init__(self, nc, n_cores):
        import jax
        from jax.sharding import Mesh, PartitionSpec
        from jax.experimental.shard_map import shard_map
        from concourse import bass2jax
        from concourse.bass2jax import _bass_exec_p, install_neuronx_cc_hook

        install_neuronx_cc_hook()
        self.jax = jax
        self.n_cores = n_cores
        partition_name = (
            nc.partition_id_tensor.name if nc.partition_id_tensor else None)
        in_names, out_names, out_avals, zero_outs = [], [], [], []
        for alloc in nc.m.functions[0].allocations:
            if not isinstance(alloc, mybir.MemoryLocationSet):
                continue
            name = alloc.memorylocations[0].name
            if alloc.kind == "ExternalInput":
                if name != partition_name:
                    in_names.append(name)
            elif alloc.kind == "ExternalOutput":
                shape = tuple(alloc.tensor_shape)
                dtype = mybir.dt.np(alloc.dtype)
                out_names.append(name)
                out_avals.append(jax.core.ShapedArray(shape, dtype))
                zero_outs.append(np.zeros(shape, dtype))
        self.in_names, self.out_names = in_names, out_names
        self.out_avals, self.zero_outs = out_avals, zero_outs
        all_in = list(in_names) + list(out_names)
        if partition_name is not None:
            all_in.append(partition_name)

        def _body(*args):
            operands = list(args)
            if partition_name is not None:
                operands.append(bass2jax.partition_id_tensor())
            return tuple(_bass_exec_p.bind(
                *operands,
                out_avals=tuple(out_avals),
                in_names=tuple(all_in),
                out_names=tuple(out_names),
                lowering_input_output_aliases=(),
                sim_require_finite=True,
                sim_require_nnan=True,
                nc=nc,
            ))

        devices = jax.devices()[:n_cores]
        self.mesh = Mesh(np.asarray(devices), ("core",))
        n_outs = len(out_names)
        self.fn = jax.jit(
            shard_map(_body, mesh=self.mesh,
                      in_specs=(PartitionSpec("core"),) * (len(in_names) + n_outs),
                      out_specs=(PartitionSpec("core"),) * n_outs,
                      check_rep=False),
            keep_unused=True,
        )

    def stage(self, in_maps):
        from jax.sharding import NamedSharding, PartitionSpec
        concat = [np.concatenate([np.asarray(m[n]) for m in in_maps], axis=0)
                  for n in self.in_names]
        concat += [np.zeros((self.n_cores * z.shape[0], *z.shape[1:]), z.dtype)
                   for z in self.zero_outs]
        sh = NamedSharding(self.mesh, PartitionSpec("core"))
        staged = [self.jax.device_put(a, sh) for a in concat]
        self.jax.block_until_ready(staged)
        return staged

    def run_staged(self, staged):
        outs = self.fn(*staged)
        self.jax.block_until_ready(outs)
        return outs

    def split(self, outs):
        return [
            {n: np.asarray(outs[i]).reshape(self.n_cores,
                                            *self.out_avals[i].shape)[c]
             for i, n in enumerate(self.out_names)}
            for c in range(self.n_cores)
        ]


_CACHE = {}


def _get_runner(cfg_key, cfg, lay):
    if cfg_key not in _CACHE:
        nc = build_nc(cfg, lay)
        _CACHE[cfg_key] = _Runner(nc, cfg.C)
    return _CACHE[cfg_key]


def kernel(node_features, edge_index, edge_label_index, w_neigh, w_self,
           bias):
    node_features = np.asarray(node_features)
    edge_index = np.asarray(edge_index)
    edge_label_index = np.asarray(edge_label_index)
    N = node_features.shape[0]
    C = 8
    NPCR = N // C
    ELC = edge_label_index.shape[1] // C
    cfg = Cfg(C, NPCR, ELC)
    lay = Layout(cfg, edge_index, edge_label_index)
    runner = _get_runner((C, NPCR, ELC, lay.key()), cfg, lay)
    in_maps, perm = prep_inputs(cfg, lay, node_features, edge_index,
                                edge_label_index, w_neigh, w_self, bias)
    outs = runner.split(runner.run_staged(runner.stage(in_maps)))
    lcol, lpart = perm
    EL = edge_label_index.shape[1]
    lc = np.arange(EL) // ELC
    res = np.empty(EL, np.float32)
    for c in range(C):
        m = lc == c
        res[m] = outs[c]["logits"][lcol[m], lpart[m]]
    return res.astype(np.float32)


# revision 10
# speedup vs baseline: 37.0300x; 26.1137x over previous
"""GraphSAGE link predictor on 8 Trainium2 NeuronCores (Bass/Tile).

Strategy (graph/data parallel, from the sharding hint):
- Nodes are sharded contiguously across 8 cores (12500 real -> 12544 padded
  per core, 98 dst tiles of 128). Edges are sharded by dst node. Features
  travel in bf16 (fp32 accumulation in PSUM); ~3e-3 max rel err vs fp32.
- The full row-major bf16 h replica lives in DRAM (layer 0: host-supplied x;
  later: AllGather of per-core slabs). Edge-source rows are gathered with the
  gpsimd dma_gather ucode (thousands of rows per Pool dispatch vs 128 for
  indirect_dma_start). Its int16 indices cap the table at 32K rows, so the
  replica is split into 4 windows of NPAD/4 rows; per (4-dst-tile chunk,
  window) one gather call fetches every needed source row. Slots are laid out
  [window][tile] with 16-aligned per-(tile,window) capacities (max over cores
  for SPMD uniformity), so a 128-slot block may straddle two adjacent tiles:
  those boundary blocks get a second one-hot (dlocB) and a second matmul.
- Per 128-edge block: one-hot matmul (lhsT=onehot[edge,dst], rhs=rows) accum
  agg[dst,feat] in PSUM; degree-normalize via per-partition ACT scale (bf16);
  PE-transpose to aggT[feat,dst]; combine matmuls (wn^T aggT + ws^T hT
  [+ I hT]) + bias/relu; transpose back to rows, chunked DMA to the slab,
  AllGather -> replica for the next layer.
- Decoder: label-edge pairs grouped by (src window, dst window) into 16
  groups; per group two dma_gathers (h3[src], h3[dst]) + DVE mul/reduce ->
  logits; the host inverts the slot permutation.
All weights ([128,128]) are replicated to every core.
"""

import numpy as np

import concourse.bacc as bacc
import concourse.mybir as mybir
import concourse.tile as tile
from concourse.masks import make_identity

P = 128
D = 128
NW = 4                    # index windows over the padded node table
MAXI = 1024               # max idxs per dma_gather call (desc-ring capacity)
F32 = mybir.dt.float32
BF16 = mybir.dt.bfloat16
I16 = mybir.dt.int16


def _bf(a):
    import ml_dtypes
    return np.asarray(a).astype(ml_dtypes.bfloat16)


def _r16(x):
    return (x + 15) // 16 * 16


class Cfg:
    def __init__(self, C, NPCR, ELC):
        self.C = C
        self.NPCR = NPCR
        self.TPC = (NPCR + P - 1) // P
        self.NPC = self.TPC * P
        self.NPAD = C * self.NPC
        self.ELC = ELC
        assert self.NPAD % NW == 0
        self.WROWS = self.NPAD // NW
        assert self.WROWS <= 32767


class Layout:
    """Core-uniform slot layout (compile-time structure)."""

    def __init__(self, cfg, edge_index, edge_label_index):
        C, TPC, NPCR, NPC, WROWS = (cfg.C, cfg.TPC, cfg.NPCR, cfg.NPC,
                                    cfg.WROWS)
        src = np.asarray(edge_index[0], dtype=np.int64)
        dst = np.asarray(edge_index[1], dtype=np.int64)
        src_pidx = (src // NPCR) * NPC + (src % NPCR)
        win = src_pidx // WROWS
        c_of = dst // NPCR
        loc = dst - c_of * NPCR
        t_of = loc // P
        # counts per (core, tile, window) -> capacities (max over cores, r16)
        cnt = np.zeros((C * TPC, NW), np.int64)
        np.add.at(cnt, (c_of * TPC + t_of, win), 1)
        cnt = cnt.reshape(C, TPC, NW)
        caps = _r16(cnt.max(axis=0))
        caps = np.where((caps > 0) & (caps < P), P, caps)
        dead = caps.sum(axis=1) == 0
        caps[dead, 0] = P            # tiles with no edges anywhere: 1 pad blk
        self.caps = caps

        n_chunk = (TPC + 3) // 4
        self.n_chunk = n_chunk
        self.S = np.zeros((n_chunk, NW), np.int64)      # idxs per (chunk,win)
        self.wb0 = np.zeros((n_chunk, NW), np.int64)    # block base in chunk
        self.BCH = np.zeros(n_chunk, np.int64)          # blocks per chunk
        self.qbase = np.zeros((TPC, NW), np.int64)      # q offset of (t,k)
        for ch in range(n_chunk):
            t0, t1 = ch * 4, min(ch * 4 + 4, TPC)
            nb = 0
            for k in range(NW):
                q = 0
                for t in range(t0, t1):
                    self.qbase[t, k] = q
                    q += caps[t, k]
                self.S[ch, k] = q
                self.wb0[ch, k] = nb
                nb += (q + P - 1) // P
            self.BCH[ch] = nb
        self.BMAX = int(self.BCH.max())
        self.O = np.zeros(n_chunk + 1, np.int64)        # global block col base
        np.cumsum(self.BCH, out=self.O[1:])
        self.TOTB = int(self.O[-1])
        # idx col offsets per (chunk, win)
        self.oc = np.zeros((n_chunk, NW), np.int64)
        o = 0
        for ch in range(n_chunk):
            for k in range(NW):
                self.oc[ch, k] = o
                o += self.S[ch, k] // 16
        self.TOTC = int(o)
        self.ICMAX = int(max(self.S[ch].sum() // 16 for ch in range(n_chunk)))

        # primary/secondary tile per global block col + boundary packing
        self.primary = np.zeros(self.TOTB, np.int64)
        self.bnd_col = np.full(self.TOTB, -1, np.int64)  # packed dlocB col
        sched = []                                      # [ch][t-local] ops
        nbb = 0
        self.bnd_of_chunk = []
        for ch in range(n_chunk):
            t0, t1 = ch * 4, min(ch * 4 + 4, TPC)
            ops = [[] for _ in range(t1 - t0)]
            ch_bnd = 0
            for k in range(NW):
                # tile of flat q (within chunk,win)
                edges = []
                for t in range(t0, t1):
                    edges += [t] * caps[t, k]
                edges = np.asarray(edges, np.int64)
                for b in range((self.S[ch, k] + P - 1) // P):
                    j = self.O[ch] + self.wb0[ch, k] + b
                    q0, q1 = b * P, min(b * P + P, self.S[ch, k])
                    tset = edges[q0:q1]
                    tp = int(tset[0])
                    self.primary[j] = tp
                    ops[tp - t0].append(("A", int(j)))
                    if int(tset[-1]) != tp:
                        ts = int(tset[-1])
                        assert ts == tp + 1, (tp, ts)
                        self.bnd_col[j] = nbb + ch_bnd
                        ops[ts - t0].append(("B", int(nbb + ch_bnd)))
                        ch_bnd += 1
            self.bnd_of_chunk.append((nbb, ch_bnd))
            nbb += ch_bnd
            sched.append(ops)
        self.sched = sched
        self.TOTBB = max(int(nbb), 1)
        self.NBMAX = max(max(b for _, b in self.bnd_of_chunk), 1)

        # ---- decoder groups ----
        lsrc = np.asarray(edge_label_index[0], dtype=np.int64)
        ldst = np.asarray(edge_label_index[1], dtype=np.int64)
        ELC = cfg.ELC
        lc = np.arange(lsrc.shape[0]) // ELC
        ls_p = (lsrc // NPCR) * NPC + (lsrc % NPCR)
        ld_p = (ldst // NPCR) * NPC + (ldst % NPCR)
        gg = (ls_p // WROWS) * NW + (ld_p // WROWS)
        gcnt = np.zeros((C, NW * NW), np.int64)
        np.add.at(gcnt, (lc, gg), 1)
        self.dcaps = _r16(gcnt.max(axis=0))
        self.dblocks = (self.dcaps + P - 1) // P
        self.Gbase = np.zeros(NW * NW + 1, np.int64)
        np.cumsum(self.dblocks, out=self.Gbase[1:])
        self.LBLKP = int(self.Gbase[-1])
        self.DGBMAX = int(self.dblocks.max())
        # decoder idx col offsets (se then de per group), after main idx cols
        self.doc = np.zeros((NW * NW, 2), np.int64)
        o = self.TOTC
        for g in range(NW * NW):
            self.doc[g, 0] = o
            o += self.dcaps[g] // 16
            self.doc[g, 1] = o
            o += self.dcaps[g] // 16
        self.TOTALC = int(o)

    def key(self):
        import hashlib
        h = hashlib.sha1()
        h.update(self.caps.tobytes())
        h.update(self.dcaps.tobytes())
        return h.hexdigest()


def build_nc(cfg, lay, n_layers=3, scratch=16384):
    C, TPC, NPC, NPAD, WROWS = (cfg.C, cfg.TPC, cfg.NPC, cfg.NPAD, cfg.WROWS)
    n_chunk, BMAX, NBMAX = lay.n_chunk, lay.BMAX, lay.NBMAX
    LBLKP, DGBMAX = lay.LBLKP, lay.DGBMAX

    nc = bacc.Bacc("TRN2", target_bir_lowering=False, debug=False,
                   num_devices=C, dynamic_dma_scratch_size=scratch,
                   num_swdge_queues=4)

    # ---- I/O ----
    xfull = nc.dram_tensor("xfull", [NPAD, D], BF16, kind="ExternalInput")
    xt = nc.dram_tensor("xt", [P, NPC], BF16, kind="ExternalInput")
    idxs = nc.dram_tensor("idxs", [P, lay.TOTALC], I16, kind="ExternalInput")
    dlocA = nc.dram_tensor("dlocA", [P, lay.TOTB], BF16, kind="ExternalInput")
    dlocB = nc.dram_tensor("dlocB", [P, lay.TOTBB], BF16,
                           kind="ExternalInput")
    invd = nc.dram_tensor("invd", [P, TPC], F32, kind="ExternalInput")
    wn_d = nc.dram_tensor("wn", [3, D, D], BF16, kind="ExternalInput")
    ws_d = nc.dram_tensor("ws", [3, D, D], BF16, kind="ExternalInput")
    bias_d = nc.dram_tensor("bias", [3, D], F32, kind="ExternalInput")
    logits = nc.dram_tensor("logits", [LBLKP, P], F32, kind="ExternalOutput")

    slabs = [nc.dram_tensor(f"slab{l}", [NPC, D], BF16, kind="Internal")
             for l in range(3)]
    reps = [nc.dram_tensor(f"rep{l}", [NPAD, D], BF16, kind="Internal",
                           addr_space="Shared") for l in range(3)]

    with tile.TileContext(nc) as tc:
        with (
            tc.tile_pool(name="big", bufs=1) as bigp,
            tc.tile_pool(name="const", bufs=1) as cstp,
            tc.tile_pool(name="oh", bufs=1) as ohp,
            tc.tile_pool(name="xe", bufs=1) as xep,
            tc.tile_pool(name="aggsb", bufs=1) as asbp,
            tc.tile_pool(name="rows", bufs=1) as rowp,
            tc.tile_pool(name="dec", bufs=1) as decp,
            tc.tile_pool(name="psA", bufs=4, space="PSUM") as psA,
            tc.tile_pool(name="psT", bufs=2, space="PSUM") as psT,
            tc.tile_pool(name="psC", bufs=2, space="PSUM") as psC,
        ):
            hA = bigp.tile([P, NPC], BF16, name="hA")
            hB = bigp.tile([P, NPC], BF16, name="hB")
            dlocA_sb = cstp.tile([P, lay.TOTB], BF16, name="dlocA_sb")
            dlocB_sb = cstp.tile([P, lay.TOTBB], BF16, name="dlocB_sb")
            invd_sb = cstp.tile([P, TPC], F32, name="invd_sb")
            bias_sb = cstp.tile([P, 3], F32, name="bias_sb")
            ident_b = cstp.tile([P, D], BF16, name="ident_b")
            ident_f = cstp.tile([P, D], F32, name="ident_f")
            iota_b = cstp.tile([P, D], BF16, name="iota_b")
            wcst = cstp.tile([P, 6 * D], BF16, name="wcst")

            nc.sync.dma_start(out=hA[:], in_=xt.ap())
            nc.sync.dma_start(out=dlocA_sb[:], in_=dlocA.ap())
            nc.sync.dma_start(out=dlocB_sb[:], in_=dlocB.ap())
            nc.sync.dma_start(out=invd_sb[:], in_=invd.ap())

            wn_t = [wcst[:, l * D:(l + 1) * D] for l in range(3)]
            ws_t = [wcst[:, (3 + l) * D:(4 + l) * D] for l in range(3)]
            make_identity(nc, ident_b)
            make_identity(nc, ident_f)
            iota_i = cstp.tile([P, D], mybir.dt.int32, name="iota_i")
            nc.gpsimd.iota(iota_i[:], pattern=[[1, D]], base=0,
                           channel_multiplier=0)
            nc.vector.tensor_copy(iota_b, iota_i[:])
            for l in range(3):
                nc.sync.dma_start(out=wn_t[l], in_=wn_d.ap()[l])
                nc.sync.dma_start(out=ws_t[l], in_=ws_d.ap()[l])
                nc.sync.dma_start(out=bias_sb[:, l:l + 1],
                                  in_=bias_d.ap()[l][:, None])

            xe_big = xep.tile([P, 2 * BMAX * D], BF16, name="xe_big")
            ohA_big = ohp.tile([P, 2 * BMAX * D], BF16, name="ohA_big")
            ohB_big = ohp.tile([P, 2 * NBMAX * D], BF16, name="ohB_big")
            idx_sb = cstp.tile([P, 2 * lay.ICMAX], I16, name="idx_sb")
            aggn = asbp.tile([P, 2 * D], BF16, name="aggn")
            aggT = asbp.tile([P, 2 * 512], BF16, name="aggT")
            rows_sb = rowp.tile([P, 2 * 4 * D], BF16, name="rows_sb")
            nc.vector.memset(xe_big[:], 0.0)

            slab_v = [slabs[l].ap().rearrange("(g p) d -> p g d", p=P)
                      for l in range(3)]

            # ================= 3 GraphSAGE layers =================
            for l in range(n_layers):
                h_in = hA if l % 2 == 0 else hB
                h_out = hB if l % 2 == 0 else hA
                src_t = xfull if l == 0 else reps[l - 1]

                for ch in range(n_chunk):
                    t0 = ch * 4
                    tiles = list(range(t0, min(t0 + 4, TPC)))
                    cn = len(tiles) * P
                    sl = ch % 2
                    # stage this chunk's idx cols
                    c0 = int(lay.oc[ch, 0])
                    ccols = int(lay.S[ch].sum() // 16)
                    ix = idx_sb[:, sl * lay.ICMAX:sl * lay.ICMAX + ccols]
                    nc.sync.dma_start(out=ix, in_=idxs.ap()[:, c0:c0 + ccols])
                    # one gather per window (split at MAXI idxs per call)
                    xe_c = xe_big[:, sl * BMAX * D:(sl + 1) * BMAX * D]
                    for k in range(NW):
                        S = int(lay.S[ch, k])
                        if S == 0:
                            continue
                        b0 = int(lay.wb0[ch, k])
                        o0 = int(lay.oc[ch, k]) - c0
                        for s0 in range(0, S, MAXI):
                            ns = min(MAXI, S - s0)
                            nb_k = (ns + P - 1) // P
                            bb = b0 + s0 // P
                            nc.gpsimd.dma_gather(
                                out_ap=xe_c[:, bb * D:(bb + nb_k) * D]
                                    .rearrange("p (s d) -> p s d", d=D),
                                in_ap=src_t.ap()[k * WROWS:(k + 1) * WROWS],
                                idxs_ap=ix[:, o0 + s0 // 16:
                                           o0 + s0 // 16 + ns // 16],
                                num_idxs=ns, num_idxs_reg=ns, elem_size=D,
                                queue_num=k % 4)
                    # one-hots
                    BCH = int(lay.BCH[ch])
                    ohA_c = ohA_big[:, sl * BMAX * D:sl * BMAX * D + BCH * D]
                    nc.vector.tensor_tensor(
                        out=ohA_c.rearrange("p (b d) -> p b d", b=BCH),
                        in0=dlocA_sb[:, lay.O[ch]:lay.O[ch] + BCH][:, :, None]
                            .broadcast_to([P, BCH, D]),
                        in1=iota_b[:, None, :].broadcast_to([P, BCH, D]),
                        op=mybir.AluOpType.is_equal,
                    )
                    bb0, nbnd = lay.bnd_of_chunk[ch]
                    if nbnd:
                        ohB_c = ohB_big[:, sl * NBMAX * D:
                                        sl * NBMAX * D + nbnd * D]
                        nc.vector.tensor_tensor(
                            out=ohB_c.rearrange("p (b d) -> p b d", b=nbnd),
                            in0=dlocB_sb[:, bb0:bb0 + nbnd][:, :, None]
                                .broadcast_to([P, nbnd, D]),
                            in1=iota_b[:, None, :].broadcast_to([P, nbnd, D]),
                            op=mybir.AluOpType.is_equal,
                        )
                    aggT_c = aggT[:, sl * 512:sl * 512 + cn]
                    for kk, t in enumerate(tiles):
                        ops = lay.sched[ch][kk]
                        agg_ps = psA.tile([P, D], F32, tag="agg",
                                          name=f"agg_{l}_{t}")
                        for j, (which, col) in enumerate(ops):
                            if which == "A":
                                oh = ohA_c[:, (col - int(lay.O[ch])) * D:
                                           (col - int(lay.O[ch]) + 1) * D]
                                xe = xe_c[:, (col - int(lay.O[ch])) * D:
                                          (col - int(lay.O[ch]) + 1) * D]
                            else:
                                oh = ohB_c[:, (col - bb0) * D:
                                           (col - bb0 + 1) * D]
                                # find the A col of this boundary block
                                gcol = int(np.where(lay.bnd_col == col)[0][0])
                                xe = xe_c[:, (gcol - int(lay.O[ch])) * D:
                                          (gcol - int(lay.O[ch]) + 1) * D]
                            nc.tensor.matmul(
                                out=agg_ps[:], lhsT=oh, rhs=xe,
                                start=(j == 0), stop=(j == len(ops) - 1),
                            )
                        agg_n = aggn[:, (t % 2) * D:((t % 2) + 1) * D]
                        nc.scalar.activation(
                            out=agg_n, in_=agg_ps[:],
                            func=mybir.ActivationFunctionType.Copy,
                            scale=invd_sb[:, t:t + 1],
                        )
                        tps = psT.tile([P, D], BF16, tag="tp",
                                       name=f"tp_{l}_{t}")
                        nc.tensor.transpose(out=tps[:], in_=agg_n,
                                            identity=ident_b)
                        nc.vector.tensor_copy(aggT_c[:, kk * P:(kk + 1) * P],
                                              tps[:])
                    # combine
                    cs = t0 * P
                    cps = psC.tile([P, 512], F32, tag="comb",
                                   name=f"cb_{l}_{ch}")
                    nc.tensor.matmul(out=cps[:, :cn], lhsT=wn_t[l],
                                     rhs=aggT_c, start=True, stop=False)
                    nc.tensor.matmul(out=cps[:, :cn], lhsT=ws_t[l],
                                     rhs=h_in[:, cs:cs + cn],
                                     start=False, stop=l == 0)
                    if l > 0:
                        nc.tensor.matmul(out=cps[:, :cn], lhsT=ident_b,
                                         rhs=h_in[:, cs:cs + cn],
                                         start=False, stop=True)
                    if l < 2:
                        nc.scalar.activation(
                            out=h_out[:, cs:cs + cn], in_=cps[:, :cn],
                            func=mybir.ActivationFunctionType.Relu,
                            bias=bias_sb[:, l:l + 1],
                        )
                    else:
                        nc.vector.tensor_scalar_add(
                            out=h_out[:, cs:cs + cn], in0=cps[:, :cn],
                            scalar1=bias_sb[:, l:l + 1],
                        )
                    rsb = rows_sb[:, sl * 4 * D:sl * 4 * D + len(tiles) * D]
                    for kk, t in enumerate(tiles):
                        rps = psT.tile([P, D], BF16, tag="tp",
                                       name=f"rw_{l}_{t}")
                        nc.tensor.transpose(
                            out=rps[:], in_=h_out[:, t * P:(t + 1) * P],
                            identity=ident_b)
                        nc.scalar.activation(
                            out=rsb[:, kk * D:(kk + 1) * D], in_=rps[:],
                            func=mybir.ActivationFunctionType.Copy)
                    nc.sync.dma_start(
                        out=slab_v[l][:, t0:t0 + len(tiles), :],
                        in_=rsb.rearrange("p (g d) -> p g d", g=len(tiles)))
                nc.gpsimd.collective_compute(
                    "AllGather", mybir.AluOpType.bypass,
                    replica_groups=[list(range(C))],
                    ins=[slabs[l].ap()], outs=[reps[l].ap()],
                )

            # ================= link decoder =================
            se_big = decp.tile([P, 2 * DGBMAX * D], BF16, name="se_big")
            de_big = decp.tile([P, 2 * DGBMAX * D], BF16, name="de_big")
            didx = decp.tile([P, 2 * 2 * (DGBMAX * P // 16)], I16,
                             name="didx")
            pr = decp.tile([P, DGBMAX * D], F32, name="pr")
            resL = rowp.tile([P, LBLKP], F32, name="resL")
            nc.vector.memset(se_big[:], 0.0)
            nc.vector.memset(de_big[:], 0.0)
            rep3 = reps[n_layers - 1]
            for g in range(NW * NW):
                cap = int(lay.dcaps[g])
                if cap == 0:
                    continue
                gb = int(lay.dblocks[g])
                gb0 = int(lay.Gbase[g])
                sl = g % 2
                ks, kd = g // NW, g % NW
                hw = DGBMAX * P // 16
                icols = cap // 16
                ixs = didx[:, sl * 2 * hw:sl * 2 * hw + icols]
                ixd = didx[:, sl * 2 * hw + hw:sl * 2 * hw + hw + icols]
                so = int(lay.doc[g, 0])
                do = int(lay.doc[g, 1])
                nc.sync.dma_start(out=ixs, in_=idxs.ap()[:, so:so + icols])
                nc.sync.dma_start(out=ixd, in_=idxs.ap()[:, do:do + icols])
                se = se_big[:, sl * DGBMAX * D:sl * DGBMAX * D + gb * D]
                de = de_big[:, sl * DGBMAX * D:sl * DGBMAX * D + gb * D]
                for s0 in range(0, cap, MAXI):
                    ns = min(MAXI, cap - s0)
                    nb_k = (ns + P - 1) // P
                    bb = s0 // P
                    nc.gpsimd.dma_gather(
                        out_ap=se[:, bb * D:(bb + nb_k) * D]
                            .rearrange("p (s d) -> p s d", d=D),
                        in_ap=rep3.ap()[ks * WROWS:(ks + 1) * WROWS],
                        idxs_ap=ixs[:, s0 // 16:s0 // 16 + ns // 16],
                        num_idxs=ns, num_idxs_reg=ns, elem_size=D,
                        queue_num=(2 * g) % 4)
                    nc.gpsimd.dma_gather(
                        out_ap=de[:, bb * D:(bb + nb_k) * D]
                            .rearrange("p (s d) -> p s d", d=D),
                        in_ap=rep3.ap()[kd * WROWS:(kd + 1) * WROWS],
                        idxs_ap=ixd[:, s0 // 16:s0 // 16 + ns // 16],
                        num_idxs=ns, num_idxs_reg=ns, elem_size=D,
                        queue_num=(2 * g + 1) % 4)
                nc.vector.tensor_mul(out=pr[:, :gb * D], in0=se, in1=de)
                nc.vector.tensor_reduce(
                    out=resL[:, gb0:gb0 + gb],
                    in_=pr[:, :gb * D].rearrange("p (b d) -> p b d", b=gb),
                    axis=mybir.AxisListType.X, op=mybir.AluOpType.add)
            # transpose resL -> logits rows
            nb_full = LBLKP // P
            rem = LBLKP - nb_full * P
            for j in range(nb_full + (1 if rem else 0)):
                w = P if j < nb_full else rem
                tps = psT.tile([P, D], F32, tag="tp", name=f"lt_{j}")
                nc.tensor.transpose(out=tps[:w, :],
                                    in_=resL[:, j * P:j * P + w],
                                    identity=ident_f)
                lsb = rowp.tile([P, D], F32, name=f"lsb_{j}")
                nc.scalar.activation(out=lsb[:w, :], in_=tps[:w, :],
                                     func=mybir.ActivationFunctionType.Copy)
                nc.sync.dma_start(out=logits.ap()[j * P:j * P + w, :],
                                  in_=lsb[:w, :])
    nc.compile()
    return nc


# ----------------------------------------------------------------------------
# host-side preprocessing
# ----------------------------------------------------------------------------

def prep_inputs(cfg, lay, node_features, edge_index, edge_label_index,
                w_neigh, w_self, bias):
    C, NPCR, TPC, NPC, NPAD, ELC, WROWS = (
        cfg.C, cfg.NPCR, cfg.TPC, cfg.NPC, cfg.NPAD, cfg.ELC, cfg.WROWS)
    N = node_features.shape[0]

    src = np.asarray(edge_index[0], dtype=np.int64)
    dst = np.asarray(edge_index[1], dtype=np.int64)
    deg = np.bincount(dst, minlength=N).astype(np.float32)
    invdeg = 1.0 / np.maximum(deg, 1.0)

    src_pidx = (src // NPCR) * NPC + (src % NPCR)
    win = src_pidx // WROWS
    c_of = dst // NPCR
    loc = dst - c_of * NPCR
    t_of = loc // P
    d_of = loc % P

    # rank of each edge within its (core, tile, window) group
    key = (c_of * TPC + t_of) * NW + win
    order = np.argsort(key, kind="stable")
    skey = key[order]
    gcnt = np.bincount(skey, minlength=C * TPC * NW)
    starts = np.zeros(C * TPC * NW, np.int64)
    starts[1:] = np.cumsum(gcnt)[:-1]
    rank = np.arange(src.shape[0]) - starts[skey]
    # scatter rank back to edge order
    rank_e = np.empty_like(rank)
    rank_e[order] = rank

    ch_of = t_of // 4
    # flat q within (chunk, window)
    q = lay.qbase[t_of, win] + rank_e
    blk_in_ch = lay.wb0[ch_of, win] + q // P       # block col within chunk
    gblk = lay.O[ch_of] + blk_in_ch                # global block col
    part = q % P
    # idx position (within (ch,win) idx cols)
    icol = lay.oc[ch_of, win] + q // 16
    irow = q % 16

    idx_val = (src_pidx - win * WROWS).astype(np.int16)
    prim = lay.primary[gblk]
    is_prim = prim == t_of
    bcol = lay.bnd_col[gblk]

    idxs_pc = np.zeros((C, P, lay.TOTALC), np.int16)
    dlocA_pc = np.full((C, P, lay.TOTB), -1.0, np.float32)
    dlocB_pc = np.full((C, P, lay.TOTBB), -1.0, np.float32)
    for c in range(C):
        m = c_of == c
        for g in range(8):
            idxs_pc[c, 16 * g + irow[m] % 16, icol[m]] = idx_val[m]
        mp = m & is_prim
        dlocA_pc[c, part[mp], gblk[mp]] = d_of[mp]
        ms = m & ~is_prim
        assert (bcol[ms] >= 0).all()
        dlocB_pc[c, part[ms], bcol[ms]] = d_of[ms]

    # padded x replica (bf16)
    x = np.asarray(node_features, dtype=np.float32)
    xpad = np.zeros((NPAD, D), np.float32)
    for c in range(C):
        xpad[c * NPC:c * NPC + NPCR] = x[c * NPCR:(c + 1) * NPCR]
    xpad = _bf(xpad)

    invd_pc = np.ones((C, P, TPC), np.float32)
    for c in range(C):
        v = np.ones(NPC, np.float32)
        v[:NPCR] = invdeg[c * NPCR:(c + 1) * NPCR]
        invd_pc[c] = v.reshape(TPC, P).T

    # ---- decoder ----
    lsrc = np.asarray(edge_label_index[0], dtype=np.int64)
    ldst = np.asarray(edge_label_index[1], dtype=np.int64)
    lc = np.arange(lsrc.shape[0]) // ELC
    ls_p = (lsrc // NPCR) * NPC + (lsrc % NPCR)
    ld_p = (ldst // NPCR) * NPC + (ldst % NPCR)
    ks = ls_p // WROWS
    kd = ld_p // WROWS
    gg = ks * NW + kd
    gkey = lc * (NW * NW) + gg
    gorder = np.argsort(gkey, kind="stable")
    sgkey = gkey[gorder]
    ggcnt = np.bincount(sgkey, minlength=C * NW * NW)
    gstarts = np.zeros(C * NW * NW, np.int64)
    gstarts[1:] = np.cumsum(ggcnt)[:-1]
    grank = np.arange(lsrc.shape[0]) - gstarts[sgkey]
    grank_e = np.empty_like(grank)
    grank_e[gorder] = grank

    # slot -> resL position: col = Gbase[g] + r//128, partition = r%128
    lcol = lay.Gbase[gg] + grank_e // P
    lpart = grank_e % P
    # host-side inverse perm: label i of core c reads logits[lcol, lpart]
    perm = (lcol, lpart)

    se_val = (ls_p - ks * WROWS).astype(np.int16)
    de_val = (ld_p - kd * WROWS).astype(np.int16)
    sicol = lay.doc[gg, 0] + grank_e // 16
    dicol = lay.doc[gg, 1] + grank_e // 16
    for c in range(C):
        m = lc == c
        for g in range(8):
            rr = 16 * g + grank_e[m] % 16
            idxs_pc[c, rr, sicol[m]] = se_val[m]
            idxs_pc[c, rr, dicol[m]] = de_val[m]

    wn = _bf(np.ascontiguousarray(np.asarray(w_neigh, dtype=np.float32)))
    ws = _bf(np.ascontiguousarray(np.asarray(w_self, dtype=np.float32)))
    bs = np.ascontiguousarray(np.asarray(bias, dtype=np.float32))

    in_maps = []
    for c in range(C):
        xtc = np.zeros((P, NPC), np.float32)
        xtc[:, :NPCR] = x[c * NPCR:(c + 1) * NPCR].T
        in_maps.append({
            "xfull": xpad,
            "xt": _bf(xtc),
            "idxs": np.ascontiguousarray(idxs_pc[c]),
            "dlocA": _bf(dlocA_pc[c]),
            "dlocB": _bf(dlocB_pc[c]),
            "invd": np.ascontiguousarray(invd_pc[c]),
            "wn": wn, "ws": ws, "bias": bs,
        })
    return in_maps, perm


# ----------------------------------------------------------------------------
# PJRT runner (inlined; kernel.py must be self-contained)
# ----------------------------------------------------------------------------

class _Runner:
    def __init__(self, nc, n_cores):
        import jax
        from jax.sharding import Mesh, PartitionSpec
        from jax.experimental.shard_map import shard_map
        from concourse import bass2jax
        from concourse.bass2jax import _bass_exec_p, install_neuronx_cc_hook

        install_neuronx_cc_hook()
        self.jax = jax
        self.n_cores = n_cores
        partition_name = (
            nc.partition_id_tensor.name if nc.partition_id_tensor else None)
        in_names, out_names, out_avals, zero_outs = [], [], [], []
        for alloc in nc.m.functions[0].allocations:
            if not isinstance(alloc, mybir.MemoryLocationSet):
                continue
            name = alloc.memorylocations[0].name
            if alloc.kind == "ExternalInput":
                if name != partition_name:
                    in_names.append(name)
            elif alloc.kind == "ExternalOutput":
                shape = tuple(alloc.tensor_shape)
                dtype = mybir.dt.np(alloc.dtype)
                out_names.append(name)
                out_avals.append(jax.core.ShapedArray(shape, dtype))
                zero_outs.append(np.zeros(shape, dtype))
        self.in_names, self.out_names = in_names, out_names
        self.out_avals, self.zero_outs = out_avals, zero_outs
        all_in = list(in_names) + list(out_names)
        if partition_name is not None:
            all_in.append(partition_name)

        def _body(*args):
            operands = list(args)
            if partition_name is not None:
                operands.append(bass2jax.partition_id_tensor())
            return tuple(_bass_exec_p.bind(
                *operands,
                out_avals=tuple(out_avals),
                in_names=tuple(all_in),
                out_names=tuple(out_names),
                lowering_input_output_aliases=(),
                sim_require_finite=True,
                sim_require_nnan=True,
                nc=nc,
            ))

        devices = jax.devices()[:n_cores]
        self.mesh = Mesh(np.asarray(devices), ("core",))
        n_outs = len(out_names)
        self.fn = jax.jit(
            shard_map(_body, mesh=self.mesh,
                      in_specs=(PartitionSpec("core"),) * (len(in_names) + n_outs),
                      out_specs=(PartitionSpec("core"),) * n_outs,
                      check_rep=False),
            keep_unused=True,
        )

    def stage(self, in_maps):
        from jax.sharding import NamedSharding, PartitionSpec
        concat = [np.concatenate([np.asarray(m[n]) for m in in_maps], axis=0)
                  for n in self.in_names]
        concat += [np.zeros((self.n_cores * z.shape[0], *z.shape[1:]), z.dtype)
                   for z in self.zero_outs]
        sh = NamedSharding(self.mesh, PartitionSpec("core"))
        staged = [self.jax.device_put(a, sh) for a in concat]
        self.jax.block_until_ready(staged)
        return staged

    def run_staged(self, staged):
        outs = self.fn(*staged)
        self.jax.block_until_ready(outs)
        return outs

    def split(self, outs):
        return [
            {n: np.asarray(outs[i]).reshape(self.n_cores,
                                            *self.out_avals[i].shape)[c]
             for i, n in enumerate(self.out_names)}
            for c in range(self.n_cores)
        ]


_CACHE = {}


def _get_runner(cfg_key, cfg, lay):
    if cfg_key not in _CACHE:
        nc = build_nc(cfg, lay)
        _CACHE[cfg_key] = _Runner(nc, cfg.C)
    return _CACHE[cfg_key]


def kernel(node_features, edge_index, edge_label_index, w_neigh, w_self,
           bias):
    node_features = np.asarray(node_features)
    edge_index = np.asarray(edge_index)
    edge_label_index = np.asarray(edge_label_index)
    N = node_features.shape[0]
    C = 8
    NPCR = N // C
    ELC = edge_label_index.shape[1] // C
    cfg = Cfg(C, NPCR, ELC)
    lay = Layout(cfg, edge_index, edge_label_index)
    runner = _get_runner((C, NPCR, ELC, lay.key()), cfg, lay)
    in_maps, perm = prep_inputs(cfg, lay, node_features, edge_index,
                                edge_label_index, w_neigh, w_self, bias)
    outs = runner.split(runner.run_staged(runner.stage(in_maps)))
    lcol, lpart = perm
    EL = edge_label_index.shape[1]
    lc = np.arange(EL) // ELC
    res = np.empty(EL, np.float32)
    for c in range(C):
        m = lc == c
        res[m] = outs[c]["logits"][lcol[m], lpart[m]]
    return res.astype(np.float32)
